# revision 1
# baseline (speedup 1.0000x reference)
"""Trainium2 Bass kernel for ComplexSpectralAttention.

Math note: with q = [q_r|q_i] (128 dims per head), Re(Q K^H) = q_r.k_r + q_i.k_i
is just the full 128-dim dot product q.k, and [out_r|out_i] = probs @ [v_r|v_i].
So this is standard 16-head causal attention with head_dim 128 and scale
1/sqrt(64), followed by the Wo projection.

Sharding (8 cores): 2-way data parallel over batch x 4-way tensor parallel over
heads. Core c handles batch b=c//4 and heads [4g, 4g+4) where g=c%4:
  - computes q^T,k^T (head-dim-major) and v from x[b]^T and the W column slices
  - flash-style causal attention per head in S^T = K Q^T layout: exp on ACT
    (no max subtraction needed; scores are bounded ~10), probabilities
    accumulate into out^T via PE, denominators via a ones-column matmul
  - normalizes out^T with broadcast reciprocal denominators
  - multiplies by the Wo row slice -> partial [N, C] output
Host sums the 4 partials per batch. All matmuls in bf16 with fp32 accumulate.
"""

import numpy as np
import ml_dtypes

B, N, C = 2, 2048, 1024
GC = 512          # per-core head columns (4 heads x 128)
HPC = 4           # heads per core
KC = 8            # contraction chunks of 128 over C
NT = N // 512     # 4 query chunks of 512
NKB = N // 128    # 16 key blocks of 128

_CACHE = {}

_BF16 = ml_dtypes.bfloat16


def _build_nc():
    import concourse.bacc as bacc
    import concourse.mybir as mybir
    import concourse.tile as tile

    f32 = mybir.dt.float32
    bf16 = mybir.dt.bfloat16
    Exp = mybir.ActivationFunctionType.Exp

    nc = bacc.Bacc("TRN2", target_bir_lowering=False, debug=False, num_devices=8)
    xt_d = nc.dram_tensor("xt", [C, N], bf16, kind="ExternalInput").ap()
    wq_d = nc.dram_tensor("wq", [C, GC], bf16, kind="ExternalInput").ap()
    wk_d = nc.dram_tensor("wk", [C, GC], bf16, kind="ExternalInput").ap()
    wv_d = nc.dram_tensor("wv", [C, GC], bf16, kind="ExternalInput").ap()
    wo_d = nc.dram_tensor("wo", [GC, C], bf16, kind="ExternalInput").ap()
    mask_d = nc.dram_tensor("mask", [128, N], bf16, kind="ExternalInput").ap()
    out_d = nc.dram_tensor("out", [N, C], f32, kind="ExternalOutput").ap()

    with tile.TileContext(nc) as tc:
        with (
            tc.tile_pool(name="const", bufs=1) as const,
            tc.tile_pool(name="dram", bufs=1, space="DRAM") as dram,
        ):
            xt_sb = const.tile([128, KC, N], bf16, name="xt_sb")
            nc.sync.dma_start(out=xt_sb, in_=xt_d.rearrange("(a p) n -> p a n", p=128))
            wq_sb = const.tile([128, KC, GC], bf16, name="wq_sb")
            nc.sync.dma_start(out=wq_sb, in_=wq_d.rearrange("(a p) n -> p a n", p=128))
            wk_sb = const.tile([128, KC, GC], bf16, name="wk_sb")
            nc.sync.dma_start(out=wk_sb, in_=wk_d.rearrange("(a p) n -> p a n", p=128))
            wv_sb = const.tile([128, KC, GC], bf16, name="wv_sb")
            nc.sync.dma_start(out=wv_sb, in_=wv_d.rearrange("(a p) n -> p a n", p=128))
            wo_sb = const.tile([128, HPC, C], bf16, name="wo_sb")
            nc.sync.dma_start(out=wo_sb, in_=wo_d.rearrange("(h p) n -> p h n", p=128))
            mask_sb = const.tile([128, N], bf16, name="mask_sb")
            nc.sync.dma_start(out=mask_sb, in_=mask_d)
            ones_sb = const.tile([128, 1], bf16, name="ones_sb")
            nc.vector.memset(ones_sb, 1.0)

            qt_sb = const.tile([128, HPC, N], bf16, name="qt_sb")
            kt_sb = const.tile([128, HPC, N], bf16, name="kt_sb")
            v_sb = const.tile([128, NKB, GC], bf16, name="v_sb")
            outT_sb = const.tile([128, HPC, N], bf16, name="outT_sb")
            den_dram = dram.tile([16, 512], f32, name="den_dram")
            rden_dram = dram.tile([16, 512], bf16, name="rden_dram")

            # ---- Phase 1: projections ----------------------------------
            # q^T/k^T: [128d, N] per head = (W chunk)^T @ x^T chunk
            # v:       [128tok, GC]      = (x^T chunk)^T @ W chunk
            with tc.tile_pool(name="ps1", bufs=4, space="PSUM") as ps1:
                for wsb, dst in ((wq_sb, qt_sb), (wk_sb, kt_sb)):
                    for h in range(HPC):
                        for t in range(NT):
                            acc = ps1.tile([128, 512], f32, tag="proj", name="acc")
                            for kc in range(KC):
                                nc.tensor.matmul(
                                    acc,
                                    wsb[:, kc, h * 128 : (h + 1) * 128],
                                    xt_sb[:, kc, t * 512 : (t + 1) * 512],
                                    start=(kc == 0),
                                    stop=(kc == KC - 1),
                                )
                            nc.vector.tensor_copy(
                                dst[:, h, t * 512 : (t + 1) * 512], acc
                            )
                for t in range(NKB):
                    acc = ps1.tile([128, 512], f32, tag="proj", name="acc")
                    for kc in range(KC):
                        nc.tensor.matmul(
                            acc,
                            xt_sb[:, kc, t * 128 : (t + 1) * 128],
                            wv_sb[:, kc, :],
                            start=(kc == 0),
                            stop=(kc == KC - 1),
                        )
                    nc.vector.tensor_copy(v_sb[:, t, :], acc)

            # ---- Phase 2: attention per (head, query-chunk) ------------
            with (
                tc.tile_pool(name="psS", bufs=2, space="PSUM") as psS,
                tc.tile_pool(name="psO", bufs=2, space="PSUM") as psO,
                tc.tile_pool(name="psD", bufs=2, space="PSUM") as psD,
                tc.tile_pool(name="ptp", bufs=3) as ptp,
                tc.tile_pool(name="dstage", bufs=2) as dstage,
            ):
                for h in range(HPC):
                    for t in range(NT):
                        acc_o = psO.tile([128, 512], f32, name="acc_o")
                        acc_d = psD.tile([1, 512], f32, name="acc_d")
                        nkb = 4 * t + 4  # key blocks in play (causal)
                        for kbg in range(0, nkb, 2):
                            s = psS.tile([128, 1024], f32, name="s")
                            for j in range(2):
                                kb = kbg + j
                                nc.tensor.matmul(
                                    s[:, j * 512 : (j + 1) * 512],
                                    kt_sb[:, h, kb * 128 : (kb + 1) * 128],
                                    qt_sb[:, h, t * 512 : (t + 1) * 512],
                                    start=True,
                                    stop=True,
                                )
                            p = ptp.tile([128, 1024], bf16, name="p")
                            nc.scalar.activation(p, s, Exp, scale=0.125)
                            if kbg >= 4 * t:
                                # diagonal groups: zero the key>query corner
                                mc = (kbg - 4 * t) * 512
                                nc.vector.tensor_mul(p, p, mask_sb[:, mc : mc + 1024])
                            for j in range(2):
                                kb = kbg + j
                                nc.tensor.matmul(
                                    acc_o,
                                    v_sb[:, kb, h * 128 : (h + 1) * 128],
                                    p[:, j * 512 : (j + 1) * 512],
                                    start=(kb == 0),
                                    stop=(kb == nkb - 1),
                                )
                                nc.tensor.matmul(
                                    acc_d,
                                    ones_sb,
                                    p[:, j * 512 : (j + 1) * 512],
                                    start=(kb == 0),
                                    stop=(kb == nkb - 1),
                                )
                        nc.scalar.copy(outT_sb[:, h, t * 512 : (t + 1) * 512], acc_o)
                        dst = dstage.tile([1, 512], f32, name="dst")
                        nc.scalar.copy(dst, acc_d)
                        r = h * 4 + t
                        nc.sync.dma_start(out=den_dram[r : r + 1, :], in_=dst)

                # ---- Phase 2.5: normalize out^T ------------------------
                den_sb = dstage.tile([16, 512], f32, tag="den_sb", name="den_sb")
                nc.sync.dma_start(out=den_sb, in_=den_dram)
                rden = dstage.tile([16, 512], f32, tag="rden", name="rden")
                nc.vector.reciprocal(rden, den_sb)
                rden_bf = dstage.tile([16, 512], bf16, tag="rden_bf", name="rden_bf")
                nc.vector.tensor_copy(rden_bf, rden)
                nc.sync.dma_start(out=rden_dram, in_=rden_bf)
                for h in range(HPC):
                    R = ptp.tile([128, N], bf16, tag="R", bufs=2, name="R")
                    for t in range(NT):
                        r = h * 4 + t
                        nc.sync.dma_start(
                            out=R[:, t * 512 : (t + 1) * 512],
                            in_=rden_dram[r : r + 1, :].to_broadcast([128, 512]),
                        )
                    nc.vector.tensor_mul(outT_sb[:, h, :], outT_sb[:, h, :], R)

            # ---- Phase 3: Wo partial product ---------------------------
            with (
                tc.tile_pool(name="psF", bufs=2, space="PSUM") as psF,
                tc.tile_pool(name="fout", bufs=3) as fpool,
            ):
                for t in range(NKB):  # 16 chunks of 128 tokens
                    acc = psF.tile([128, C], f32, name="acc")
                    for n2 in range(2):
                        for h in range(HPC):
                            nc.tensor.matmul(
                                acc[:, n2 * 512 : (n2 + 1) * 512],
                                outT_sb[:, h, t * 128 : (t + 1) * 128],
                                wo_sb[:, h, n2 * 512 : (n2 + 1) * 512],
                                start=(h == 0),
                                stop=(h == HPC - 1),
                            )
                    fo = fpool.tile([128, C], f32, name="fo")
                    nc.scalar.copy(fo, acc)
                    nc.sync.dma_start(out=out_d[t * 128 : (t + 1) * 128, :], in_=fo)

    nc.compile()
    return nc


def _get_nc():
    if "nc" not in _CACHE:
        _CACHE["nc"] = _build_nc()
    return _CACHE["nc"]


def _make_mask():
    # mask[:, j*512:(j+1)*512][i, q] = 1.0 iff q >= 128*j + i  (j = kb offset
    # within the diagonal 4-key-block group, q = query offset within chunk)
    i = np.arange(128)[:, None]
    qv = np.arange(512)[None, :]
    cols = [(qv >= 128 * j + i) for j in range(4)]
    return np.concatenate(cols, axis=1).astype(_BF16)


def make_in_maps(x, Wq, Wk, Wv, Wo):
    mask = _make_mask()
    in_maps = []
    for c in range(8):
        b, g = divmod(c, 4)
        in_maps.append(
            {
                "xt": np.ascontiguousarray(x[b].T).astype(_BF16),
                "wq": np.ascontiguousarray(Wq[:, g * GC : (g + 1) * GC]).astype(_BF16),
                "wk": np.ascontiguousarray(Wk[:, g * GC : (g + 1) * GC]).astype(_BF16),
                "wv": np.ascontiguousarray(Wv[:, g * GC : (g + 1) * GC]).astype(_BF16),
                "wo": np.ascontiguousarray(Wo[g * GC : (g + 1) * GC, :]).astype(_BF16),
                "mask": mask,
            }
        )
    return in_maps


def gather_out(results):
    out = np.zeros((B, N, C), np.float32)
    for c in range(8):
        out[c // 4] += results[c]["out"]
    return out


def kernel(x, Wq, Wk, Wv, Wo):
    from concourse.bass_utils import run_bass_kernel_spmd

    nc = _get_nc()
    in_maps = make_in_maps(
        np.asarray(x, np.float32),
        np.asarray(Wq, np.float32),
        np.asarray(Wk, np.float32),
        np.asarray(Wv, np.float32),
        np.asarray(Wo, np.float32),
    )
    res = run_bass_kernel_spmd(nc, in_maps, core_ids=list(range(8)))
    return gather_out(res.results)
